# revision 9
# baseline (speedup 1.0000x reference)
"""Trainium2 Bass kernel for CoordLSVotingWeighted (segment_reduce).

Strategy: data-parallel over batch B=8 across 8 NeuronCores (1 image/core).
Per image, on device:
  - hard one-hot of argmax over 9 seg channels (matches softmax(seg*1e6))
  - unit-direction projection matrices R = w*(I - n n^T), w = softplus(w)
  - segment-reduce per class via TensorE matmul:
      psum[24,27] = sum_pix lhsT[pix, {hot, hot*ch, hot*cw}]^T
                    @ rhs[pix, {R00, m, R11}]   (m = -R01 = w*nx*ny/s)
Host: assemble 2x2 systems in float64, pinv-solve, scale by HEIGHT.

Self-contained: only needs numpy / ml_dtypes / concourse (installed env).
"""

import os

import numpy as np

B = 8
H = 128
W = 128
NCLS = 9  # seg channels, class 0 = background
NPTS = 9
OC = 8
HEIGHT = 128.0
N_CORES = 8

_cache: dict = {}


def _build_nc():
    import concourse.bacc as bacc
    import concourse.tile as tile
    import concourse.mybir as mybir
    from concourse.alu_op_type import AluOpType as Alu

    Act = mybir.ActivationFunctionType
    Axis = mybir.AxisListType
    f32 = mybir.dt.float32
    b16 = mybir.dt.bfloat16

    nc = bacc.Bacc(
        "TRN2", target_bir_lowering=False, debug=False, num_devices=N_CORES
    )
    seg_d = nc.dram_tensor("seg", [H, W * NCLS], f32, kind="ExternalInput")
    dct_d = nc.dram_tensor("direct", [H, W * NPTS * 2], f32, kind="ExternalInput")
    w_d = nc.dram_tensor("w", [H, W * NPTS], f32, kind="ExternalInput")
    cw_d = nc.dram_tensor("cw8", [H, OC * W], b16, kind="ExternalInput")
    ch_d = nc.dram_tensor("chv", [H, 1], f32, kind="ExternalInput")
    out_d = nc.dram_tensor("acc", [3 * OC, 3 * NPTS], f32, kind="ExternalOutput")

    NF = W * NPTS  # 1152

    with tile.TileContext(nc) as tc:
        with (
            tc.tile_pool(name="main", bufs=1) as pool,
            tc.tile_pool(name="ps", bufs=1, space="PSUM") as psp,
        ):
            # ---- input tiles
            sgt = pool.tile([H, W * NCLS], f32, tag="sgt")
            dct = pool.tile([H, W * NPTS * 2], f32, tag="dct")
            wdt = pool.tile([H, W * NPTS], f32, tag="wdt")
            cwt = pool.tile([H, OC * W], b16, tag="cwt")
            cht = pool.tile([H, 1], f32, tag="cht")
            # two DMA queues in parallel: {w, seg, ch} on sync, {direct, cw} on vector
            nc.sync.dma_start(out=wdt[:, :], in_=w_d[:, :])
            nc.scalar.dma_start(out=dct[:, :], in_=dct_d[:, :])
            nc.sync.dma_start(out=sgt[:, :], in_=seg_d[:, :])
            nc.scalar.dma_start(out=cwt[:, :], in_=cw_d[:, :])
            nc.sync.dma_start(out=cht[:, :], in_=ch_d[:, :])

            # ---- work tiles (bf16 unless noted)
            nxy = pool.tile([H, 2 * NF], b16, tag="nxy")   # [x|y] p-major planes
            sq = pool.tile([H, 2 * NF], b16, tag="sq")     # [x^2|y^2]
            s16 = pool.tile([H, NF], b16, tag="s16")
            ls32 = pool.tile([H, NF], f32, tag="ls32")
            rr16 = pool.tile([H, NF], b16, tag="rr16")
            ew16 = pool.tile([H, NF], b16, tag="ew16")
            sp16 = pool.tile([H, NF], b16, tag="sp16")
            k16 = pool.tile([H, NF], b16, tag="k16")
            u16 = pool.tile([H, NF], b16, tag="u16")
            mx = pool.tile([H, W], f32, tag="mx")
            b9 = pool.tile([H, 1], f32, tag="b9")
            nc.vector.memset(b9[:, :], 1e-9)
            # dummy Exp with no data deps: pulls the single ln/exp ACT table
            # load to t=0 so it overlaps the input DMAs
            warm = pool.tile([H, 1], f32, tag="warm")
            nc.scalar.activation(out=warm[:, :], in_=b9[:, :], func=Act.Exp)
            L = pool.tile([H, 3 * OC * W], b16, tag="L")   # hot|hot*ch|hot*cw
            R = pool.tile([H, 3 * NF], b16, tag="R")       # R00|m|R11
            outs = pool.tile([3 * OC, 3 * NPTS], f32, tag="outs")

            # ---- direction repack: direct[w,(p,xy)] -> nxy[(xy,p,w)] (ACT copy)
            dct_r = dct[:, :].rearrange("q (w g t) -> q t g w", g=NPTS, t=2)
            nxy_r = nxy[:, :].rearrange("q (t g w) -> q t g w", t=2, g=NPTS)
            nc.gpsimd.tensor_copy(out=nxy_r, in_=dct_r)

            # ---- softplus(w) = Ln(Exp(w) + 1), single ACT table set (ln/exp)
            w_r = wdt[:, :].rearrange("q (w g) -> q g w", g=NPTS)
            ew_r = ew16[:, :].rearrange("q (g w) -> q g w", g=NPTS)
            nc.scalar.activation(out=ew_r, in_=w_r, func=Act.Exp)
            nc.scalar.activation(out=sp16[:, :], in_=ew16[:, :], func=Act.Ln, bias=1.0)

            # ---- squares / s / 1/s
            nc.vector.tensor_tensor(
                out=sq[:, :], in0=nxy[:, :], in1=nxy[:, :], op=Alu.mult
            )
            nc.vector.tensor_tensor(
                out=s16[:, :], in0=sq[:, 0:NF], in1=sq[:, NF : 2 * NF], op=Alu.add
            )
            nc.scalar.activation(
                out=ls32[:, :], in_=s16[:, :], func=Act.Ln, bias=b9[:, :]
            )
            nc.scalar.activation(out=rr16[:, :], in_=ls32[:, :], func=Act.Exp, scale=-1.0)

            # ---- k = softplus(w)/s ; rhs features R00=k*ny^2, m=k*nx*ny, R11=k*nx^2
            nc.vector.tensor_tensor(
                out=k16[:, :], in0=sp16[:, :], in1=rr16[:, :], op=Alu.mult
            )
            nc.vector.tensor_tensor(
                out=R[:, 0:NF], in0=k16[:, :], in1=sq[:, NF : 2 * NF], op=Alu.mult
            )
            nc.vector.tensor_tensor(
                out=u16[:, :], in0=k16[:, :], in1=nxy[:, 0:NF], op=Alu.mult
            )
            nc.vector.tensor_tensor(
                out=R[:, NF : 2 * NF], in0=u16[:, :], in1=nxy[:, NF : 2 * NF],
                op=Alu.mult,
            )
            nc.vector.tensor_tensor(
                out=R[:, 2 * NF : 3 * NF], in0=k16[:, :], in1=sq[:, 0:NF], op=Alu.mult
            )

            # ---- one-hot lhs: hot = (seg_c == max_c seg), classes 1..8
            sgt_wc = sgt[:, :].rearrange("q (w c) -> q w c", c=NCLS)
            nc.vector.tensor_reduce(
                out=mx[:, :], in_=sgt_wc, axis=Axis.X, op=Alu.max
            )
            sgt_cw = sgt[:, :].rearrange("q (w c) -> q c w", c=NCLS)[:, 1:NCLS, :]
            mx_b = mx[:, :].unsqueeze(1).broadcast_to((H, OC, W))
            hot_r = L[:, 0 : OC * W].rearrange("q (c w) -> q c w", c=OC)
            nc.vector.tensor_tensor(
                out=hot_r, in0=sgt_cw, in1=mx_b, op=Alu.is_equal
            )
            nc.vector.tensor_scalar_mul(
                L[:, OC * W : 2 * OC * W], L[:, 0 : OC * W], cht[:, :]
            )
            nc.vector.tensor_tensor(
                out=L[:, 2 * OC * W : 3 * OC * W], in0=L[:, 0 : OC * W],
                in1=cwt[:, :], op=Alu.mult,
            )

            # ---- segment reduce: 128 accumulating matmuls over w-chunks
            acc = psp.tile([3 * OC, 3 * NPTS], f32, tag="acc")
            for wi in range(W):
                nc.tensor.matmul(
                    acc[:, :],
                    L[:, wi :: W],
                    R[:, wi :: W],
                    start=(wi == 0),
                    stop=(wi == W - 1),
                )

            nc.vector.tensor_copy(out=outs[:, :], in_=acc[:, :])
            nc.sync.dma_start(out=out_d[:, :], in_=outs[:, :])

    nc.compile()
    return nc


def _host_constants():
    import ml_dtypes

    bf16 = ml_dtypes.bfloat16
    coord = ((np.arange(128, dtype=np.float32) + 0.5) / HEIGHT).astype(bf16)
    cw8 = np.ascontiguousarray(
        np.broadcast_to(coord[None, None, :], (H, OC, W))
    ).reshape(H, OC * W)
    chv = ((np.arange(128, dtype=np.float32) + 0.5) / HEIGHT).reshape(H, 1)
    return cw8, chv


def _solve_host(acc_f32: np.ndarray) -> np.ndarray:
    """acc [24,27] fp32 -> p [OC, NPTS, 2] fp32 (float64 pinv like reference)."""
    a = acc_f32.astype(np.float64)
    A = a[0:OC, 0:9]
    Bm = a[0:OC, 9:18]
    D = a[0:OC, 18:27]
    S1 = a[OC : 2 * OC, 0:9]
    S3 = a[OC : 2 * OC, 9:18]
    S2 = a[2 * OC : 3 * OC, 9:18]
    S4 = a[2 * OC : 3 * OC, 18:27]
    Rm = np.empty((OC, NPTS, 2, 2), dtype=np.float64)
    Rm[..., 0, 0] = A
    Rm[..., 0, 1] = -Bm
    Rm[..., 1, 0] = -Bm
    Rm[..., 1, 1] = D
    q = np.stack([S1 - S2, S4 - S3], axis=-1)
    Rp = np.linalg.pinv(Rm.reshape(-1, 2, 2)).reshape(Rm.shape)
    p = np.einsum("cpij,cpj->cpi", Rp, q) * HEIGHT
    return p.astype(np.float32)


def kernel(seg, direct, w):
    if "nc" not in _cache:
        _cache["nc"] = _build_nc()
    nc = _cache["nc"]

    seg = np.ascontiguousarray(np.asarray(seg, dtype=np.float32))
    direct = np.ascontiguousarray(np.asarray(direct, dtype=np.float32))
    w = np.ascontiguousarray(np.asarray(w, dtype=np.float32))
    cw8, chv = _host_constants()

    in_maps = []
    for i in range(B):
        in_maps.append(
            {
                "seg": seg[i].reshape(H, W * NCLS),
                "direct": direct[i].reshape(H, W * NPTS * 2),
                "w": w[i].reshape(H, W * NPTS),
                "cw8": cw8,
                "chv": chv,
            }
        )

    from concourse.bass_utils import run_bass_kernel_spmd

    trace = bool(int(os.environ.get("KERNEL_TRACE", "0")))
    res = run_bass_kernel_spmd(
        nc, in_maps, core_ids=list(range(N_CORES)), trace=trace
    )
    kernel._last_exec_ns = res.exec_time_ns
    kernel._last_results = res

    out = np.stack(
        [_solve_host(np.asarray(res.results[i]["acc"])) for i in range(B)], axis=0
    )
    return out


# revision 13
# speedup vs baseline: 1.0163x; 1.0163x over previous
"""Trainium2 Bass kernel for CoordLSVotingWeighted (segment_reduce).

Strategy: data-parallel over batch B=8 across 8 NeuronCores (1 image/core).
Per image, on device:
  - hard one-hot of argmax over 9 seg channels (matches softmax(seg*1e6))
  - unit-direction projection matrices R = w*(I - n n^T), w = softplus(w)
  - segment-reduce per class via TensorE matmul:
      psum[24,27] = sum_pix lhsT[pix, {hot, hot*ch, hot*cw}]^T
                    @ rhs[pix, {R00, m, R11}]   (m = -R01 = w*nx*ny/s)
Host: assemble 2x2 systems in float64, pinv-solve, scale by HEIGHT.

Self-contained: only needs numpy / ml_dtypes / concourse (installed env).
"""

import os

import numpy as np

B = 8
H = 128
W = 128
NCLS = 9  # seg channels, class 0 = background
NPTS = 9
OC = 8
HEIGHT = 128.0
N_CORES = 8

_cache: dict = {}


def _build_nc():
    import concourse.bacc as bacc
    import concourse.tile as tile
    import concourse.mybir as mybir
    from concourse.alu_op_type import AluOpType as Alu

    Act = mybir.ActivationFunctionType
    Axis = mybir.AxisListType
    f32 = mybir.dt.float32
    b16 = mybir.dt.bfloat16

    nc = bacc.Bacc(
        "TRN2", target_bir_lowering=False, debug=False, num_devices=N_CORES
    )
    seg_d = nc.dram_tensor("seg", [H, W * NCLS], f32, kind="ExternalInput")
    dct_d = nc.dram_tensor("direct", [H, W * NPTS * 2], f32, kind="ExternalInput")
    w_d = nc.dram_tensor("w", [H, W * NPTS], f32, kind="ExternalInput")
    cw_d = nc.dram_tensor("cw8", [H, OC * W], b16, kind="ExternalInput")
    ch_d = nc.dram_tensor("chv", [H, 1], f32, kind="ExternalInput")
    out_d = nc.dram_tensor("acc", [3 * OC, 3 * NPTS], f32, kind="ExternalOutput")

    NF = W * NPTS  # 1152

    with tile.TileContext(nc) as tc:
        with (
            tc.tile_pool(name="main", bufs=1) as pool,
            tc.tile_pool(name="ps", bufs=1, space="PSUM") as psp,
        ):
            # ---- input tiles
            sgt = pool.tile([H, W * NCLS], f32, tag="sgt")
            dct = pool.tile([H, W * NPTS * 2], f32, tag="dct")
            wdt = pool.tile([H, W * NPTS], f32, tag="wdt")
            cwt = pool.tile([H, OC * W], b16, tag="cwt")
            cht = pool.tile([H, 1], f32, tag="cht")
            # two DMA queues in parallel: {w, seg, ch} on sync, {direct, cw} on vector
            nc.sync.dma_start(out=wdt[:, :], in_=w_d[:, :])
            nc.scalar.dma_start(out=dct[:, :], in_=dct_d[:, :])
            nc.sync.dma_start(out=sgt[:, :], in_=seg_d[:, :])
            nc.scalar.dma_start(out=cwt[:, :], in_=cw_d[:, :])
            nc.sync.dma_start(out=cht[:, :], in_=ch_d[:, :])

            # ---- work tiles (bf16 unless noted)
            sq = pool.tile([H, 2 * NF], b16, tag="sq")     # [x^2|y^2]
            s16 = pool.tile([H, NF], b16, tag="s16")
            ls32 = pool.tile([H, NF], f32, tag="ls32")
            rr16 = pool.tile([H, NF], b16, tag="rr16")
            ew16 = pool.tile([H, NF], b16, tag="ew16")
            sp16 = pool.tile([H, NF], b16, tag="sp16")
            k16 = pool.tile([H, NF], b16, tag="k16")
            u16 = pool.tile([H, NF], b16, tag="u16")
            mx = pool.tile([H, W], f32, tag="mx")
            b9 = pool.tile([H, 1], f32, tag="b9")
            nc.vector.memset(b9[:, :], 1e-9)
            # dummy Exp with no data deps: pulls the single ln/exp ACT table
            # load to t=0 so it overlaps the input DMAs
            warm = pool.tile([H, 1], f32, tag="warm")
            nc.scalar.activation(out=warm[:, :], in_=b9[:, :], func=Act.Exp)
            L = pool.tile([H, 3 * OC * W], b16, tag="L")   # hot|hot*ch|hot*cw
            R = pool.tile([H, 3 * NF], b16, tag="R")       # R00|m|R11
            outs = pool.tile([3 * OC, 3 * NPTS], f32, tag="outs")

            # ---- one-hot lhs first: depends only on seg DMA
            sgt_wc = sgt[:, :].rearrange("q (w c) -> q w c", c=NCLS)
            nc.vector.tensor_reduce(
                out=mx[:, :], in_=sgt_wc, axis=Axis.X, op=Alu.max
            )
            sgt_cw = sgt[:, :].rearrange("q (w c) -> q c w", c=NCLS)[:, 1:NCLS, :]
            mx_b = mx[:, :].unsqueeze(1).broadcast_to((H, OC, W))
            hot_r = L[:, 0 : OC * W].rearrange("q (c w) -> q c w", c=OC)
            nc.vector.tensor_tensor(
                out=hot_r, in0=sgt_cw, in1=mx_b, op=Alu.is_equal
            )
            nc.vector.tensor_scalar_mul(
                L[:, OC * W : 2 * OC * W], L[:, 0 : OC * W], cht[:, :]
            )
            nc.vector.tensor_tensor(
                out=L[:, 2 * OC * W : 3 * OC * W], in0=L[:, 0 : OC * W],
                in1=cwt[:, :], op=Alu.mult,
            )

            # ---- strided views of direct: nx = even cols, ny = odd cols
            nxs = dct[:, 0::2].rearrange("q (w g) -> q g w", g=NPTS)
            nys = dct[:, 1::2].rearrange("q (w g) -> q g w", g=NPTS)

            # ---- softplus(w) = Ln(Exp(w) + 1), single ACT table set (ln/exp)
            w_r = wdt[:, :].rearrange("q (w g) -> q g w", g=NPTS)
            ew_r = ew16[:, :].rearrange("q (g w) -> q g w", g=NPTS)
            nc.scalar.activation(out=ew_r, in_=w_r, func=Act.Exp)
            nc.scalar.activation(out=sp16[:, :], in_=ew16[:, :], func=Act.Ln, bias=1.0)

            # ---- squares via ACT (Square is in the resident table set)
            sqx_r = sq[:, 0:NF].rearrange("q (g w) -> q g w", g=NPTS)
            sqy_r = sq[:, NF : 2 * NF].rearrange("q (g w) -> q g w", g=NPTS)
            nc.scalar.activation(out=sqx_r, in_=nxs, func=Act.Square)
            nc.scalar.activation(out=sqy_r, in_=nys, func=Act.Square)
            nc.vector.tensor_tensor(
                out=s16[:, :], in0=sq[:, 0:NF], in1=sq[:, NF : 2 * NF], op=Alu.add
            )
            nc.scalar.activation(
                out=ls32[:, :], in_=s16[:, :], func=Act.Ln, bias=b9[:, :]
            )
            nc.scalar.activation(out=rr16[:, :], in_=ls32[:, :], func=Act.Exp, scale=-1.0)

            # ---- k = softplus(w)/s ; rhs features R00=k*ny^2, m=k*nx*ny, R11=k*nx^2
            nc.vector.tensor_tensor(
                out=k16[:, :], in0=sp16[:, :], in1=rr16[:, :], op=Alu.mult
            )
            nc.vector.tensor_tensor(
                out=R[:, 0:NF], in0=k16[:, :], in1=sq[:, NF : 2 * NF], op=Alu.mult
            )
            k16_r = k16[:, :].rearrange("q (g w) -> q g w", g=NPTS)
            u16_r = u16[:, :].rearrange("q (g w) -> q g w", g=NPTS)
            nc.vector.tensor_tensor(out=u16_r, in0=k16_r, in1=nxs, op=Alu.mult)
            nc.vector.tensor_tensor(
                out=R[:, NF : 2 * NF].rearrange("q (g w) -> q g w", g=NPTS),
                in0=u16_r, in1=nys, op=Alu.mult,
            )
            nc.vector.tensor_tensor(
                out=R[:, 2 * NF : 3 * NF], in0=k16[:, :], in1=sq[:, 0:NF], op=Alu.mult
            )

            # ---- segment reduce: 128 accumulating matmuls over w-chunks
            acc = psp.tile([3 * OC, 3 * NPTS], f32, tag="acc")
            for wi in range(W):
                nc.tensor.matmul(
                    acc[:, :],
                    L[:, wi :: W],
                    R[:, wi :: W],
                    start=(wi == 0),
                    stop=(wi == W - 1),
                )

            nc.vector.tensor_copy(out=outs[:, :], in_=acc[:, :])
            nc.sync.dma_start(out=out_d[:, :], in_=outs[:, :])

    nc.compile()
    return nc


def _host_constants():
    import ml_dtypes

    bf16 = ml_dtypes.bfloat16
    coord = ((np.arange(128, dtype=np.float32) + 0.5) / HEIGHT).astype(bf16)
    cw8 = np.ascontiguousarray(
        np.broadcast_to(coord[None, None, :], (H, OC, W))
    ).reshape(H, OC * W)
    chv = ((np.arange(128, dtype=np.float32) + 0.5) / HEIGHT).reshape(H, 1)
    return cw8, chv


def _solve_host(acc_f32: np.ndarray) -> np.ndarray:
    """acc [24,27] fp32 -> p [OC, NPTS, 2] fp32 (float64 pinv like reference)."""
    a = acc_f32.astype(np.float64)
    A = a[0:OC, 0:9]
    Bm = a[0:OC, 9:18]
    D = a[0:OC, 18:27]
    S1 = a[OC : 2 * OC, 0:9]
    S3 = a[OC : 2 * OC, 9:18]
    S2 = a[2 * OC : 3 * OC, 9:18]
    S4 = a[2 * OC : 3 * OC, 18:27]
    Rm = np.empty((OC, NPTS, 2, 2), dtype=np.float64)
    Rm[..., 0, 0] = A
    Rm[..., 0, 1] = -Bm
    Rm[..., 1, 0] = -Bm
    Rm[..., 1, 1] = D
    q = np.stack([S1 - S2, S4 - S3], axis=-1)
    Rp = np.linalg.pinv(Rm.reshape(-1, 2, 2)).reshape(Rm.shape)
    p = np.einsum("cpij,cpj->cpi", Rp, q) * HEIGHT
    return p.astype(np.float32)


def kernel(seg, direct, w):
    if "nc" not in _cache:
        _cache["nc"] = _build_nc()
    nc = _cache["nc"]

    seg = np.ascontiguousarray(np.asarray(seg, dtype=np.float32))
    direct = np.ascontiguousarray(np.asarray(direct, dtype=np.float32))
    w = np.ascontiguousarray(np.asarray(w, dtype=np.float32))
    cw8, chv = _host_constants()

    in_maps = []
    for i in range(B):
        in_maps.append(
            {
                "seg": seg[i].reshape(H, W * NCLS),
                "direct": direct[i].reshape(H, W * NPTS * 2),
                "w": w[i].reshape(H, W * NPTS),
                "cw8": cw8,
                "chv": chv,
            }
        )

    from concourse.bass_utils import run_bass_kernel_spmd

    trace = bool(int(os.environ.get("KERNEL_TRACE", "0")))
    res = run_bass_kernel_spmd(
        nc, in_maps, core_ids=list(range(N_CORES)), trace=trace
    )
    kernel._last_exec_ns = res.exec_time_ns
    kernel._last_results = res

    out = np.stack(
        [_solve_host(np.asarray(res.results[i]["acc"])) for i in range(B)], axis=0
    )
    return out
